# revision 1
# baseline (speedup 1.0000x reference)
"""Trainium2 Bass kernel for nn_CFModel (KGAT-style GNN message passing).

Sharding: edges partitioned by dst-node range across 8 cores (aggregation is
core-local); node features replicated; one AllGather of h_nb between layers.

Per core, per layer:
  - windows of 128 dst nodes (49/core). For each window and relation:
    V[p, r, :] = W_r @ tanh(W_r h_p + e_r)  via dense matmuls from resident
    transposed h (no dst-side gathers, relation_table folded in).
  - edge tiles (128 edge slots, 24 tiles/window = 3 statically relation-pure
    tile slots per relation, SPMD-static graph): one-hot expansion matmuls
    pick v_e = V[dst_e, rel_e]; att_e = <h_src_e, v_e> (DVE); softmax without
    max-subtraction (logits bounded ~O(1)); binning matmuls accumulate
    [sum ex*h_src | sum ex] into window PSUM; h_nb = hU / max(s, eps).
  - h_src rows: layer 1 host-staged (input preprocessing); layer 2 staged by
    a For_i loop of indirect-DMA gathers from the AllGather'd h_nb table into
    DRAM (bounded SWDGE footprint — straight-line gather streams >~500
    instructions crash the exec unit), then consumed with direct DMA.
  - All 49 V windows per layer are precomputed in a separate PSUM pool before
    the edge phase (and L2's before the gather loop), so the edge-phase
    pipeline runs with double-buffered PSUM and no window-phase contention.
Epilogue: out_i = lrelu((h+h_nb) W1_i^T) + lrelu((h*h_nb) W2_i^T), computed on
own rows; output is [h0 | out1 | out2] row-slice per core.
"""

import os
import sys

import numpy as np

sys.path.insert(0, "/opt/trn_rl_repo")

import ml_dtypes  # noqa: E402

from concourse import bacc, bass, mybir, tile  # noqa: E402
from concourse.bass_utils import run_bass_kernel_spmd  # noqa: E402

# ---------------- configuration ----------------
NCORES = 8
D = 128
R = 8
L = 2
NEG_SLOPE = 0.01

# structure constants (env-overridable for small-scale dev testing)
NPC = int(os.environ.get("GNN_NPC", 6250))          # real nodes per core
WINDOWS = int(os.environ.get("GNN_WINDOWS", 49))    # 128-node windows per core
TPW = int(os.environ.get("GNN_TPW", 24))            # tiles per window
RELSLOT = TPW // R                                  # rel-pure tile slots
GROUP = int(os.environ.get("GNN_GROUP", 6))         # tiles per group
NLOC = WINDOWS * 128
TILES = WINDOWS * TPW
GPW = TPW // GROUP
GROUPS = WINDOWS * GPW
assert TPW % GROUP == 0

BF16 = mybir.dt.bfloat16
F32 = mybir.dt.float32
I32 = mybir.dt.int32

PE = mybir.EngineType.PE
AF = mybir.ActivationFunctionType
OP = mybir.AluOpType

_CACHE = {}


def _build_nc():
    nc = bacc.Bacc("TRN2", target_bir_lowering=False, debug=False,
                   num_devices=NCORES)

    dp = nc.declare_dram_parameter
    hs1_ext = dp("hs1", [GROUPS, 128, GROUP, D], F32, isOutput=False)
    h0locT_ext = dp("h0locT", [D, NLOC], BF16, isOutput=False)
    h0loc_ext = dp("h0loc", [NLOC, D], F32, isOutput=False)
    idx2_ext = dp("idx2", [GROUPS, 128, GROUP], I32, isOutput=False)
    GUNROLL = 24
    idx2t_ext = dp("idx2t", [TILES // GUNROLL, GUNROLL, 128], I32, isOutput=False)
    dstA_ext = dp("dstA", [GROUPS, GROUP * 128], BF16, isOutput=False)
    dstF_ext = dp("dstF", [GROUPS, 128, GROUP], BF16, isOutput=False)
    Wr_ext = dp("Wr", [R, D, D], BF16, isOutput=False)      # [r][d, f]
    WrT_ext = dp("WrT", [R, D, D], BF16, isOutput=False)    # [r][f, d]
    erow_ext = dp("erow", [R, D], BF16, isOutput=False)
    W1T_ext = dp("W1T", [L, D, D], BF16, isOutput=False)    # W1.T  [j, i]
    W2T_ext = dp("W2T", [L, D, D], BF16, isOutput=False)
    out_ext = dp("out", [NLOC, 3 * D], F32, isOutput=True)

    hnb1 = nc.dram_tensor("hnb1", [NLOC, D], F32)
    hnb1bf = nc.dram_tensor("hnb1bf", [NLOC, D], BF16)
    hnb2 = nc.dram_tensor("hnb2", [NLOC, D], F32)
    hnb2bf = nc.dram_tensor("hnb2bf", [NLOC, D], BF16)
    hnb_all = nc.dram_tensor("hnb_all", [NCORES * NLOC, D], F32,
                             addr_space="Shared")
    hs2_stage = nc.dram_tensor("hs2_stage", [TILES // GUNROLL, GUNROLL, 128, D], F32)

    with tile.TileContext(nc) as tc:
        with (
            tc.tile_pool(name="const", bufs=1) as constp,
            tc.tile_pool(name="hTp", bufs=1) as hTp,
        ):
            # constants
            iota_i = constp.tile([128, 128], I32, tag="ioi")
            nc.gpsimd.iota(iota_i[:], pattern=[[1, 128]], base=0,
                           channel_multiplier=0)
            iota_bf = constp.tile([128, 128], BF16, tag="iobf")
            nc.vector.tensor_copy(out=iota_bf[:], in_=iota_i[:])
            iota_col_i = constp.tile([128, 1], I32, tag="ioci")
            nc.gpsimd.iota(iota_col_i[:], pattern=[[0, 1]], base=0,
                           channel_multiplier=1)
            iota_col = constp.tile([128, 1], F32, tag="ioc")
            nc.vector.tensor_copy(out=iota_col[:], in_=iota_col_i[:])
            ones_row = constp.tile([1, 128], BF16, tag="ones")
            nc.vector.memset(ones_row[:], 1.0)

            Wr_sb = constp.tile([D, R, D], BF16, tag="wr")       # [d, r, f]
            nc.sync.dma_start(out=Wr_sb[:],
                              in_=Wr_ext[:, :, :].rearrange("r d f -> d r f"))
            WrT_sb = constp.tile([D, R, D], BF16, tag="wrt")     # [f, r, d]
            nc.sync.dma_start(out=WrT_sb[:],
                              in_=WrT_ext[:, :, :].rearrange("r f d -> f r d"))
            erow_sb = constp.tile([1, R, D], BF16, tag="er")
            nc.sync.dma_start(out=erow_sb[:], in_=erow_ext[None, :, :])
            W1T_sb = constp.tile([D, L, D], BF16, tag="w1t")     # [j, l, i]
            nc.sync.dma_start(out=W1T_sb[:],
                              in_=W1T_ext[:, :, :].rearrange("l j i -> j l i"))
            W2T_sb = constp.tile([D, L, D], BF16, tag="w2t")
            nc.sync.dma_start(out=W2T_sb[:],
                              in_=W2T_ext[:, :, :].rearrange("l j i -> j l i"))

            hT = hTp.tile([D, NLOC], BF16, tag="hT")

            Vp_cm = tc.tile_pool(name="Vp", bufs=WINDOWS + 1)
            Vp = Vp_cm.__enter__()

            def window_phase(li):
                if li == 0 or os.environ.get("GNN_NO_TRANSPOSE"):
                    nc.sync.dma_start(out=hT[:], in_=h0locT_ext[:, :])
                else:
                    nc.sync.dma_start_transpose(out=hT[:], in_=hnb1bf[:, :])
                V_ws = []
                with (
                    tc.tile_pool(name=f"win_ps{li}", bufs=2, space="PSUM") as win_ps,
                    tc.tile_pool(name=f"twtp{li}", bufs=2) as twtp,
                ):
                    for w in range(WINDOWS):
                        twt_ps = win_ps.tile([128, R * 128], F32, tag="wps")
                        for r in range(R):
                            sl = slice(r * 128, (r + 1) * 128)
                            nc.tensor.matmul(out=twt_ps[:, sl],
                                             lhsT=Wr_sb[:, r, :],
                                             rhs=hT[:, w * 128:(w + 1) * 128],
                                             start=True, stop=False)
                            nc.tensor.matmul(out=twt_ps[:, sl],
                                             lhsT=erow_sb[:, r, :],
                                             rhs=ones_row[:],
                                             start=False, stop=True)
                        twt_sb = twtp.tile([128, R * 128], BF16, tag="twt")
                        nc.scalar.activation(out=twt_sb[:], in_=twt_ps[:],
                                             func=AF.Tanh)
                        v_ps2 = win_ps.tile([128, R * 128], F32, tag="wps")
                        for r in range(R):
                            sl = slice(r * 128, (r + 1) * 128)
                            nc.tensor.matmul(out=v_ps2[:, sl],
                                             lhsT=twt_sb[:, sl],
                                             rhs=WrT_sb[:, r, :],
                                             start=True, stop=True)
                        V_w = Vp.tile([128, R * 128], BF16, tag="V")
                        nc.scalar.activation(out=V_w[:], in_=v_ps2[:],
                                             func=AF.Copy)
                        V_ws.append(V_w)
                return V_ws

            def edge_phase(li, V_ws):
                with (
                    tc.tile_pool(name=f"edge{li}", bufs=4) as edgep,
                    tc.tile_pool(name=f"bc_ps{li}", bufs=1, space="PSUM") as bc_psp,
                    tc.tile_pool(name=f"v_ps{li}", bufs=2, space="PSUM") as v_psp,
                    tc.tile_pool(name=f"hu_ps{li}", bufs=2, space="PSUM") as hu_psp,
                    tc.tile_pool(name=f"wout{li}", bufs=2) as outp,
                ):
                    for w in range(WINDOWS):
                        V_w = V_ws[w]
                        hu_ps = hu_psp.tile([128, D + 1], F32, tag="hu")

                        for gw in range(GPW):
                            g = w * GPW + gw
                            t0 = w * TPW + gw * GROUP
                            FREE = GROUP * 128
                            # hs rows
                            hs_g = edgep.tile([128, GROUP, D], F32, tag="hs")
                            if li == 0:
                                nc.sync.dma_start(out=hs_g[:],
                                                  in_=hs1_ext[g, :, :, :])
                            else:
                                t00 = w * TPW + gw * GROUP
                                flat = hs2_stage[:, :, :, :].rearrange(
                                    "a b p d -> (a b) p d")
                                nc.sync.dma_start(
                                    out=hs_g[:],
                                    in_=flat[t00:t00 + GROUP, :, :].rearrange(
                                        "t p d -> p t d"))
                            # dst-offset rows -> PE broadcast -> one-hots
                            dstrowA = edgep.tile([1, FREE], BF16, tag="dra")
                            nc.sync.dma_start(out=dstrowA[:], in_=dstA_ext[g:g + 1, :])
                            dstF_col = edgep.tile([128, GROUP], BF16, tag="dstf")
                            nc.sync.dma_start(out=dstF_col[:], in_=dstF_ext[g, :, :])

                            bc_ps = bc_psp.tile([128, FREE], F32, tag="bc")
                            for a0 in range(0, FREE, 512):
                                a1 = min(a0 + 512, FREE)
                                nc.tensor.matmul(out=bc_ps[:, a0:a1],
                                                 lhsT=ones_row[:],
                                                 rhs=dstrowA[:, a0:a1],
                                                 start=True, stop=True)
                            otnA = edgep.tile([128, GROUP, 128], BF16, tag="otna")
                            nc.vector.tensor_scalar(
                                out=otnA[:], in0=bc_ps[:].rearrange(
                                    "p (t e) -> p t e", e=128),
                                scalar1=iota_col[:, :1], scalar2=None,
                                op0=OP.is_equal)
                            # expansion matmuls (rel static per tile slot)
                            v_ps = v_psp.tile([128, GROUP, 128], F32, tag="vps")
                            for j in range(GROUP):
                                rel_j = ((t0 + j) % TPW) // RELSLOT
                                nc.tensor.matmul(
                                    out=v_ps[:, j, :], lhsT=otnA[:, j, :],
                                    rhs=V_w[:, rel_j * 128:(rel_j + 1) * 128],
                                    start=True, stop=True)

                            # attention
                            prod = edgep.tile([128, GROUP, D], BF16, tag="prod")
                            nc.vector.tensor_tensor(out=prod[:], in0=v_ps[:],
                                                    in1=hs_g[:], op=OP.mult)
                            att = edgep.tile([128, GROUP], F32, tag="att")
                            nc.vector.tensor_reduce(out=att[:], in_=prod[:],
                                                    axis=mybir.AxisListType.X,
                                                    op=OP.add)
                            ex = edgep.tile([128, GROUP], F32, tag="ex")
                            nc.scalar.activation(out=ex[:], in_=att[:],
                                                 func=AF.Exp)

                            # messages [ex*hs | ex]
                            msg = edgep.tile([128, GROUP, D + 1], BF16, tag="msg")
                            nc.vector.tensor_tensor(
                                out=msg[:, :, 0:D], in0=hs_g[:],
                                in1=ex[:, :, None].to_broadcast([128, GROUP, D]),
                                op=OP.mult)
                            nc.vector.tensor_copy(out=msg[:, :, D:D + 1],
                                                  in_=ex[:, :, None])

                            # binning one-hot [e, p]
                            ote = edgep.tile([128, GROUP, 128], BF16, tag="ote")
                            nc.vector.tensor_tensor(
                                out=ote[:],
                                in0=dstF_col[:, :, None].to_broadcast(
                                    [128, GROUP, 128]),
                                in1=iota_bf[:, None, :].to_broadcast(
                                    [128, GROUP, 128]),
                                op=OP.is_equal)

                            for j in range(GROUP):
                                nc.tensor.matmul(
                                    out=hu_ps[:, :], lhsT=ote[:, j, :],
                                    rhs=msg[:, j, :],
                                    start=(gw == 0 and j == 0),
                                    stop=(gw == GPW - 1 and j == GROUP - 1))

                        # window epilogue: h_nb = hU / max(s, eps)
                        hu_sb = outp.tile([128, D + 1], F32, tag="husb")
                        nc.scalar.activation(out=hu_sb[:], in_=hu_ps[:],
                                             func=AF.Copy)
                        s_cl = outp.tile([128, 1], F32, tag="scl")
                        nc.vector.tensor_scalar(out=s_cl[:],
                                                in0=hu_sb[:, D:D + 1],
                                                scalar1=1e-20, scalar2=None,
                                                op0=OP.max)
                        s_inv = outp.tile([128, 1], F32, tag="sinv")
                        nc.vector.reciprocal(out=s_inv[:], in_=s_cl[:])
                        hnb_w = outp.tile([128, D], F32, tag="hnbw")
                        nc.scalar.activation(out=hnb_w[:], in_=hu_sb[:, 0:D],
                                             func=AF.Copy, scale=s_inv[:, :1])
                        dst_f32 = hnb1 if li == 0 else hnb2
                        nc.sync.dma_start(out=dst_f32[w * 128:(w + 1) * 128, :],
                                          in_=hnb_w[:])
                        hnb_bf_w = outp.tile([128, D], BF16, tag="hnbbf")
                        nc.vector.tensor_copy(out=hnb_bf_w[:], in_=hnb_w[:])
                        dst_bf = hnb1bf if li == 0 else hnb2bf
                        nc.sync.dma_start(out=dst_bf[w * 128:(w + 1) * 128, :],
                                          in_=hnb_bf_w[:])

            V1 = window_phase(0)
            edge_phase(0, V1)
            nc.gpsimd.collective_compute(
                "AllGather", OP.bypass,
                replica_groups=[list(range(NCORES))],
                ins=[hnb1[:, :]], outs=[hnb_all[:, :]])
            # L2 window phase before the gather loop (overlaps the collective)
            V2 = window_phase(1)

            # staged layer-2 src-row gather loop (bounded SWDGE footprint)
            with (
                tc.tile_pool(name="gidx", bufs=8) as gidxp,
                tc.tile_pool(name="gbuf", bufs=8) as gbufp,
            ):
                with tc.For_i(0, TILES // GUNROLL, 1) as it:
                    for u in range(GUNROLL):
                        idx_t = gidxp.tile([128, 1], I32, tag="gidx")
                        nc.sync.dma_start(
                            out=idx_t[:],
                            in_=idx2t_ext[bass.ds(it, 1), u, :].rearrange(
                                "o p -> p o"))
                        g_t = gbufp.tile([128, D], F32, tag="gbuf")
                        nc.gpsimd.indirect_dma_start(
                            out=g_t[:], out_offset=None,
                            in_=hnb_all[:, :],
                            in_offset=bass.IndirectOffsetOnAxis(
                                ap=idx_t[:, :1], axis=0))
                        nc.sync.dma_start(
                            out=hs2_stage[bass.ds(it, 1), u, :, :], in_=g_t[:])

            if not os.environ.get("GNN_SKIP_L2"):
                edge_phase(1, V2)
            Vp_cm.__exit__(None, None, None)

            # ---------------- epilogue ----------------
            with (
                tc.tile_pool(name="ep", bufs=3) as ep,
                tc.tile_pool(name="epT", bufs=1) as epT,
                tc.tile_pool(name="ep_ps", bufs=2, space="PSUM") as ep_ps,
            ):
                # hT currently holds hnb1T (layer 2's input).  Build hnb2T.
                h0T = epT.tile([D, NLOC], BF16, tag="h0T")
                nc.sync.dma_start(out=h0T[:], in_=h0locT_ext[:, :])
                h2T = epT.tile([D, NLOC], BF16, tag="h2T")
                nc.sync.dma_start_transpose(out=h2T[:], in_=hnb2bf[:, :])

                aT = epT.tile([D, L, NLOC], BF16, tag="aT")
                nc.vector.tensor_tensor(out=aT[:, 0, :], in0=h0T[:], in1=hT[:],
                                        op=OP.add)
                nc.vector.tensor_tensor(out=aT[:, 1, :], in0=hT[:], in1=h2T[:],
                                        op=OP.add)
                mT = epT.tile([D, L, NLOC], BF16, tag="mT")
                nc.vector.tensor_tensor(out=mT[:, 0, :], in0=h0T[:], in1=hT[:],
                                        op=OP.mult)
                nc.vector.tensor_tensor(out=mT[:, 1, :], in0=hT[:], in1=h2T[:],
                                        op=OP.mult)

                for w in range(WINDOWS):
                    sl = slice(w * 128, (w + 1) * 128)
                    h0_w = ep.tile([128, D], F32, tag="h0w")
                    nc.sync.dma_start(out=h0_w[:], in_=h0loc_ext[sl, :])
                    nc.sync.dma_start(out=out_ext[sl, 0:D], in_=h0_w[:])
                    for li in range(L):
                        ps1 = ep_ps.tile([128, D], F32, tag="ps1")
                        nc.tensor.matmul(out=ps1[:], lhsT=aT[:, li, sl],
                                         rhs=W1T_sb[:, li, :],
                                         start=True, stop=True)
                        ps2 = ep_ps.tile([128, D], F32, tag="ps2")
                        nc.tensor.matmul(out=ps2[:], lhsT=mT[:, li, sl],
                                         rhs=W2T_sb[:, li, :],
                                         start=True, stop=True)
                        lr1 = ep.tile([128, D], F32, tag="lr1")
                        nc.scalar.activation(out=lr1[:], in_=ps1[:],
                                             func=AF.Lrelu, alpha=NEG_SLOPE)
                        lr2 = ep.tile([128, D], F32, tag="lr2")
                        nc.scalar.activation(out=lr2[:], in_=ps2[:],
                                             func=AF.Lrelu, alpha=NEG_SLOPE)
                        o_w = ep.tile([128, D], F32, tag="ow")
                        nc.vector.tensor_tensor(out=o_w[:], in0=lr1[:],
                                                in1=lr2[:], op=OP.add)
                        nc.sync.dma_start(
                            out=out_ext[sl, (1 + li) * D:(2 + li) * D],
                            in_=o_w[:])

    nc.compile()
    return nc


# ---------------- host preprocessing ----------------

def _host_prep(node_ids, relation_ids, src, dst, entity_table, relation_table,
               relation_W, res_fc_W, res_fc2_W):
    node_ids = np.asarray(node_ids).astype(np.int64)
    rel = np.asarray(relation_ids).astype(np.int64)
    src = np.asarray(src).astype(np.int64)
    dst = np.asarray(dst).astype(np.int64)
    entity_table = np.asarray(entity_table, dtype=np.float32)
    relation_table = np.asarray(relation_table, dtype=np.float32)
    relation_W = np.asarray(relation_W, dtype=np.float32)
    res_fc_W = np.asarray(res_fc_W, dtype=np.float32)
    res_fc2_W = np.asarray(res_fc2_W, dtype=np.float32)

    bf = ml_dtypes.bfloat16
    in_maps = []
    # shared weight tensors
    Wr_bf = relation_W.astype(bf)                       # [r, d, f]
    WrT_bf = np.ascontiguousarray(relation_W.transpose(0, 2, 1)).astype(bf)
    erow_bf = relation_table.astype(bf)
    W1T_bf = np.ascontiguousarray(res_fc_W.transpose(0, 2, 1)).astype(bf)
    W2T_bf = np.ascontiguousarray(res_fc2_W.transpose(0, 2, 1)).astype(bf)

    core_of = dst // NPC
    for c in range(NCORES):
        emask = core_of == c
        e_rel = rel[emask]
        e_src = src[emask]
        e_dst = dst[emask]
        loc = e_dst - c * NPC
        wl = loc // 128
        order = np.lexsort((e_dst, e_rel, wl))
        e_rel, e_src, loc, wl = (e_rel[order], e_src[order], loc[order],
                                 wl[order])

        slot_src = np.zeros(TILES * 128, np.int64)
        slot_off = np.full(TILES * 128, -1.0, np.float32)
        slot_rel = np.full(TILES * 128, -1, np.int64)
        for w in range(WINDOWS):
            m = wl == w
            w_src = e_src[m]
            w_off = (loc[m] - w * 128).astype(np.float32)
            w_rel = e_rel[m]
            base = w * TPW * 128
            # rel-pure static slots: rel r at tiles [r*RELSLOT, (r+1)*RELSLOT)
            for rv in range(R):
                rm = w_rel == rv
                cnt = int(rm.sum())
                cap = RELSLOT * 128
                assert cnt <= cap, f"rel slot overflow: {cnt} > {cap}"
                pos = base + rv * cap
                slot_src[pos:pos + cnt] = w_src[rm]
                slot_off[pos:pos + cnt] = w_off[rm]
                slot_rel[pos:pos + cnt] = rv

        offA = slot_off.reshape(TILES, 128)

        # per-slot staged data, laid out [GROUPS, 128, GROUP, ...]
        def to_g(x, width=None):
            # x: [TILES*128(, width)] slot-major -> [GROUPS, 128, GROUP(, w)]
            x = x.reshape(GROUPS, GROUP, 128, -1)
            return np.ascontiguousarray(x.transpose(0, 2, 1, 3))

        hs_rows = entity_table[node_ids[slot_src]]
        hs_rows[slot_rel < 0] = 0.0
        hs1 = to_g(hs_rows).astype(np.float32)          # [G, 128, GROUP, D]

        idx_rows = (slot_src // NPC) * NLOC + (slot_src % NPC)
        idx_rows[slot_rel < 0] = 0
        idx2 = to_g(idx_rows)[..., 0].astype(np.int32)  # [G, 128, GROUP]
        idx2t = idx_rows.reshape(TILES // 24, 24, 128).astype(np.int32)

        dstF = to_g(slot_off)[..., 0].astype(bf)        # [G, 128, GROUP]
        dstA = offA.reshape(GROUPS, GROUP * 128).astype(bf)

        own = node_ids[c * NPC:(c + 1) * NPC]
        h0loc = np.zeros((NLOC, D), np.float32)
        h0loc[:NPC] = entity_table[own]
        h0locT = np.ascontiguousarray(h0loc.T).astype(bf)

        in_maps.append({
            "hs1": hs1, "h0locT": h0locT, "h0loc": h0loc,
            "idx2": idx2, "idx2t": idx2t, "dstA": dstA, "dstF": dstF,
            "Wr": Wr_bf, "WrT": WrT_bf, "erow": erow_bf,
            "W1T": W1T_bf, "W2T": W2T_bf,
        })
    return in_maps


def kernel(**inputs):
    if "nc" not in _CACHE:
        _CACHE["nc"] = _build_nc()
    nc = _CACHE["nc"]
    in_maps = _host_prep(**inputs)
    res = run_bass_kernel_spmd(nc, in_maps, core_ids=list(range(NCORES)))
    out = np.concatenate([res.results[c]["out"][:NPC] for c in range(NCORES)],
                         axis=0)
    return out.astype(np.float32)



# revision 7
# speedup vs baseline: 4.6648x; 4.6648x over previous
"""Trainium2 Bass kernel for nn_CFModel (KGAT-style GNN message passing).

Sharding: edges partitioned by dst-node range across 8 cores (aggregation is
core-local); node features replicated; chunked AllGather of hnb1 between
layers.

Slot layout (data-tuned, identical across cores for SPMD): per 128-dst-node
window, slots are a sequence of 32-slot blocks tagged (relation, src-half),
low-half blocks first then high-half, capacities = max actual count over the
8 cores rounded up to 32.  Tiles of 128 slots may mix relations; the
per-relation expansion matmuls slice the host-baked fp8 one-hot lhsT by
relation run.

Per core, per layer, per window:
  - hs rows (h_src per slot): layer 1 host-staged bf16 (one DMA); layer 2
    gathered straight into SBUF from the AllGather'd bf16 node table with
    dma_gather (two passes - src table split at the int16 index limit -
    in <=1024-row chunks).
  - expansion matmuls (lhsT = host-baked fp8 one-hot otnA, rhs = V window)
    pick v_e = V[dst_e, rel_e]; att = reduce(v*hs) (bf16); ex = exp(att);
    msg = [hs*ex | ex] via per-tile 4x tensor_scalar; binning matmuls
    (lhsT = host-baked fp8 ote) accumulate [sum ex*hs | sum ex] in PSUM;
    hnb = hU / max(s, eps).
Epilogue: out_i = lrelu((h+h_nb) W1_i^T) + lrelu((h*h_nb) W2_i^T) on own
rows; output is bf16 [out1 | out2]; h0 column block is assembled on host.
"""

import numpy as np

import sys

sys.path.insert(0, "/opt/trn_rl_repo")

import ml_dtypes  # noqa: E402

from concourse import bacc, bass, mybir, tile  # noqa: E402
from concourse.bass_utils import run_bass_kernel_spmd  # noqa: E402

# ---------------- configuration ----------------
NCORES = 8
D = 128
R = 8
L = 2
NEG_SLOPE = 0.01
N = 50000
NE = 150000
NPC = 6250
WINDOWS = 49
NLOC = WINDOWS * 128          # 6272
BLK = 32                      # slot block granularity
GROUP = 6                     # tiles per edge-phase group
CHUNK = 1024                  # dma_gather rows per call
# AllGather chunks (window ranges); half split of the node table falls
# exactly between chunk 1 and 2.
CHUNK_W = [(0, 13), (13, 25), (25, 37), (37, 49)]
CHROWS = [(w1 - w0) * 128 for (w0, w1) in CHUNK_W]
CUMROWS = [0]
for _r in CHROWS:
    CUMROWS.append(CUMROWS[-1] + _r * NCORES)
HALF = CUMROWS[2]             # 25600; rows below -> pass A

BF16 = mybir.dt.bfloat16
F32 = mybir.dt.float32
FP8 = mybir.dt.float8e4
I16 = mybir.dt.int16

AF = mybir.ActivationFunctionType
OP = mybir.AluOpType

_CACHE = {}


# ---------------- host planning ----------------

def _chunk_of_window(w):
    for k, (w0, w1) in enumerate(CHUNK_W):
        if w0 <= w < w1:
            return k
    raise AssertionError(w)


def _table_row(src):
    """Global row of node `src` in the chunk-major AllGather table."""
    ci = src // NPC
    loc = src % NPC
    w = loc // 128
    k = _chunk_of_window(w)
    w0 = CHUNK_W[k][0]
    return CUMROWS[k] + ci * CHROWS[k] + (w - w0) * 128 + (loc % 128)


def _plan_layout(rel, src, dst):
    """Shared-across-cores slot layout. Returns plan dict."""
    rel = np.asarray(rel)
    src = np.asarray(src)
    dst = np.asarray(dst)
    trow = _table_row_vec(src)
    half = (trow >= HALF).astype(np.int64)
    core = dst // NPC
    loc = dst % NPC
    w = loc // 128

    cnt = np.zeros((NCORES, WINDOWS, 2, R), np.int64)
    np.add.at(cnt, (core, w, half, rel), 1)
    cap = cnt.max(axis=0)                      # [WINDOWS, 2, R]
    nblk = -(-cap // BLK)                      # ceil, blocks of 32

    # per window: low blocks (rel runs), pad to tile (4 blocks), then high
    win_blocks = []                            # list of (rel, half) per block
    TA = np.zeros(WINDOWS, np.int64)
    TW = np.zeros(WINDOWS, np.int64)
    for wi in range(WINDOWS):
        lo = []
        for r in range(R):
            lo += [(r, 0)] * int(nblk[wi, 0, r])
        while len(lo) % 4:
            lo.append((-1, 0))
        hi = []
        for r in range(R):
            hi += [(r, 1)] * int(nblk[wi, 1, r])
        while len(hi) % 4:
            hi.append((-1, 1))
        win_blocks.append(lo + hi)
        TA[wi] = len(lo) // 4
        TW[wi] = (len(lo) + len(hi)) // 4
    OT = np.zeros(WINDOWS + 1, np.int64)
    OT[1:] = np.cumsum(TW)

    # expansion run slabs: per tile, one [128,128] one-hot slab per distinct
    # relation among its 4 blocks (all-dummy tiles get one zero slab, rel 0)
    tile_runs = []       # per window: list (per tile) of list of rels
    for wi in range(WINDOWS):
        blocks = win_blocks[wi]
        per_tile = []
        for j in range(int(TW[wi])):
            rels = []
            for bi in range(4):
                r = blocks[4 * j + bi][0]
                if r >= 0 and r not in rels:
                    rels.append(r)
            if not rels:
                rels = [0]
            per_tile.append(rels)
        tile_runs.append(per_tile)
    NRW = np.asarray([sum(len(rr) for rr in pt) for pt in tile_runs])
    OTR = np.zeros(WINDOWS + 1, np.int64)
    OTR[1:] = np.cumsum(NRW)
    return {
        "win_blocks": win_blocks, "TA": TA, "TW": TW, "OT": OT,
        "TOT": int(OT[-1]), "nblk": nblk,
        "tile_runs": tile_runs, "NRW": NRW, "OTR": OTR,
        "NRTOT": int(OTR[-1]),
    }


def _table_row_vec(src):
    src = np.asarray(src)
    ci = src // NPC
    loc = src % NPC
    w = loc // 128
    k = np.zeros_like(w)
    for kk, (w0, w1) in enumerate(CHUNK_W):
        k[(w >= w0) & (w < w1)] = kk
    cum = np.asarray([CUMROWS[kk] for kk in range(4)])
    chr_ = np.asarray(CHROWS)
    w0s = np.asarray([CHUNK_W[kk][0] for kk in range(4)])
    return cum[k] + ci * chr_[k] + (w - w0s[k]) * 128 + (loc % 128)


def _host_prep(node_ids, relation_ids, src, dst, entity_table, relation_table,
               relation_W, res_fc_W, res_fc2_W):
    node_ids = np.asarray(node_ids).astype(np.int64)
    rel = np.asarray(relation_ids).astype(np.int64)
    src = np.asarray(src).astype(np.int64)
    dst = np.asarray(dst).astype(np.int64)
    entity_table = np.asarray(entity_table, dtype=np.float32)
    relation_table = np.asarray(relation_table, dtype=np.float32)
    relation_W = np.asarray(relation_W, dtype=np.float32)
    res_fc_W = np.asarray(res_fc_W, dtype=np.float32)
    res_fc2_W = np.asarray(res_fc2_W, dtype=np.float32)

    bf = ml_dtypes.bfloat16
    f8 = ml_dtypes.float8_e4m3

    plan = _plan_layout(rel, src, dst)
    TOT = plan["TOT"]
    OT = plan["OT"]
    TW = plan["TW"]
    TA = plan["TA"]
    win_blocks = plan["win_blocks"]

    # shared weights
    Wr_bf = relation_W.astype(bf)                       # [r, d, f]
    WrT_bf = np.ascontiguousarray(relation_W.transpose(0, 2, 1)).astype(bf)
    erow_bf = relation_table.astype(bf)
    W1T_bf = np.ascontiguousarray(res_fc_W.transpose(0, 2, 1)).astype(bf)
    W2T_bf = np.ascontiguousarray(res_fc2_W.transpose(0, 2, 1)).astype(bf)

    # shared one-hots are per-core (slot fill differs); computed below.
    trow_all = _table_row_vec(src)
    core_of = dst // NPC

    # per-window/per-(half,rel) slot base offsets within the window
    # block sequence -> slot index of each block
    blk_rel = []          # per window: array of rel per block (-1 dummy)
    blk_base = []         # per window: slot base of each (half, rel) region
    for wi in range(WINDOWS):
        blocks = win_blocks[wi]
        rels = np.asarray([b[0] for b in blocks])
        blk_rel.append(rels)
        base = {}
        pos = 0
        for bi, (r, h) in enumerate(blocks):
            if r >= 0 and (h, r) not in base:
                base[(h, r)] = bi * BLK
        blk_base.append(base)

    in_maps = []
    for c in range(NCORES):
        em = core_of == c
        e_rel = rel[em]
        e_src = src[em]
        e_dst = dst[em]
        e_trow = trow_all[em]
        e_half = (e_trow >= HALF).astype(np.int64)
        e_loc = e_dst - c * NPC
        e_w = e_loc // 128
        e_off = e_loc % 128

        NSLOT = TOT * 128
        slot_src = np.zeros(NSLOT, np.int64)           # global node id
        slot_trow = np.zeros(NSLOT, np.int64)
        slot_off = np.full(NSLOT, -1, np.int64)

        order = np.lexsort((e_off, e_rel, e_half, e_w))
        e_rel, e_src, e_w, e_off, e_half, e_trow = (
            e_rel[order], e_src[order], e_w[order], e_off[order],
            e_half[order], e_trow[order])
        # group by (w, half, rel) and place at region base
        keys = ((e_w * 2 + e_half) * R + e_rel)
        uniq, starts = np.unique(keys, return_index=True)
        ends = np.append(starts[1:], len(keys))
        for key, s0, s1 in zip(uniq, starts, ends):
            r = int(key % R)
            h = int((key // R) % 2)
            wi = int(key // (2 * R))
            base = OT[wi] * 128 + blk_base[wi][(h, r)]
            n = s1 - s0
            cap_slots = 0  # assert capacity
            slot_src[base:base + n] = e_src[s0:s1]
            slot_trow[base:base + n] = e_trow[s0:s1]
            slot_off[base:base + n] = e_off[s0:s1]

        # layout transform: slot i of window w -> [partition i%128, tile i//128]
        # hs1: [128, TOT, 128] bf16
        hrows = entity_table[node_ids[slot_src]].astype(bf)
        hrows[slot_off < 0] = 0
        hs1 = np.ascontiguousarray(
            hrows.reshape(TOT, 128, D).transpose(1, 0, 2))

        # one-hots fp8: otnA run slabs [128(off), NRTOT, 128(slot)];
        # ote [128(slot), TOT, 128(bin)]
        offs = slot_off.reshape(TOT, 128)
        ote = np.zeros((128, TOT, 128), f8)
        tt, pp = np.nonzero(offs >= 0)
        oo = offs[tt, pp]
        ote[pp, tt, oo] = 1.0

        NRTOT = plan["NRTOT"]
        OTR = plan["OTR"]
        tile_runs = plan["tile_runs"]
        slot_relv = np.full(NSLOT, -1, np.int64)
        for wi in range(WINDOWS):
            rels = np.asarray([b[0] for b in win_blocks[wi]])
            s0 = int(OT[wi]) * 128
            slot_relv[s0:s0 + len(rels) * BLK] = np.repeat(rels, BLK)
        otnA = np.zeros((128, NRTOT, 128), f8)
        for wi in range(WINDOWS):
            rpos = int(OTR[wi])
            for j, rels in enumerate(tile_runs[wi]):
                tg = int(OT[wi]) + j
                for r in rels:
                    pmask = (offs[tg] >= 0) & (
                        slot_relv[tg * 128:(tg + 1) * 128] == r)
                    pp2 = np.nonzero(pmask)[0]
                    otnA[offs[tg, pp2], rpos, pp2] = 1.0
                    rpos += 1

        # idx int16 [128, 8*TOT]: per window, pass A tiles then pass B tiles;
        # idx j of a pass at [16-wrap], replicated 8x down partitions.
        tr = slot_trow.copy()
        tr[slot_off < 0] = 0
        trA = np.where(tr < HALF, tr, 0)
        trB = np.where(tr >= HALF, tr - HALF, 0)
        idx16 = np.zeros((128, 8 * TOT), np.int16)
        for wi in range(WINDOWS):
            s0 = int(OT[wi]) * 128
            nA = int(TA[wi]) * 128
            nW = int(TW[wi]) * 128
            vals = np.concatenate([trA[s0:s0 + nA], trB[s0 + nA:s0 + nW]])
            blkv = vals.reshape(-1, 16).T.astype(np.int16)   # [16, nW/16]
            col0 = int(OT[wi]) * 8
            for rr in range(8):
                idx16[16 * rr:16 * (rr + 1), col0:col0 + nW // 16] = blkv

        own = node_ids[c * NPC:(c + 1) * NPC]
        h0loc = np.zeros((NLOC, D), np.float32)
        h0loc[:NPC] = entity_table[own]
        h0locT = np.ascontiguousarray(h0loc.T).astype(bf)

        in_maps.append({
            "hs1": hs1, "h0locT": h0locT,
            "otnA": otnA.view(np.uint8), "ote": ote.view(np.uint8),
            "idx16": idx16,
            "Wr": Wr_bf, "WrT": WrT_bf, "erow": erow_bf,
            "W1T": W1T_bf, "W2T": W2T_bf,
        })

    # verify capacity: counts never exceed block capacity
    nblk = plan["nblk"]
    cap_slots = nblk * BLK
    # (cap built from max over cores, so guaranteed)

    return plan, in_maps


# ---------------- device program ----------------

def _build_nc(plan):
    TW = plan["TW"]
    TA = plan["TA"]
    OT = plan["OT"]
    TOT = plan["TOT"]
    blk_rel_all = [np.asarray([b[0] for b in bl]) for bl in plan["win_blocks"]]
    TMAX = int(TW.max())

    nc = bacc.Bacc("TRN2", target_bir_lowering=False, debug=False,
                   num_devices=NCORES, dynamic_dma_scratch_size=32768)

    dp = nc.declare_dram_parameter
    hs1_ext = dp("hs1", [128, TOT, 128], BF16, isOutput=False)
    h0locT_ext = dp("h0locT", [D, NLOC], BF16, isOutput=False)
    NRTOT = plan["NRTOT"]
    OTR = plan["OTR"]
    tile_runs = plan["tile_runs"]
    NRW = plan["NRW"]
    NRMAX = int(NRW.max())
    otnA_ext = dp("otnA", [128, NRTOT, 128], FP8, isOutput=False)
    ote_ext = dp("ote", [128, TOT, 128], FP8, isOutput=False)
    idx_ext = dp("idx16", [128, 8 * TOT], I16, isOutput=False)
    Wr_ext = dp("Wr", [R, D, D], BF16, isOutput=False)      # [r][d, f]
    WrT_ext = dp("WrT", [R, D, D], BF16, isOutput=False)    # [r][f, d]
    erow_ext = dp("erow", [R, D], BF16, isOutput=False)
    W1T_ext = dp("W1T", [L, D, D], BF16, isOutput=False)
    W2T_ext = dp("W2T", [L, D, D], BF16, isOutput=False)
    out_ext = dp("out12", [NLOC, 2 * D], BF16, isOutput=True)

    hnb1c = [nc.dram_tensor(f"hnb1c{k}", [CHROWS[k], D], BF16)
             for k in range(4)]
    hnb2bf = nc.dram_tensor("hnb2bf", [NLOC, D], BF16)
    hnb_all = nc.dram_tensor("hnb_all", [CUMROWS[-1], D], BF16,
                             addr_space="Shared")

    with tile.TileContext(nc) as tc:
        with (
            tc.tile_pool(name="const", bufs=1) as constp,
            tc.tile_pool(name="hTp", bufs=1) as hTp,
        ):
            Wr_sb = constp.tile([D, R, D], BF16, tag="wr")       # [d, r, f]
            nc.sync.dma_start(out=Wr_sb[:],
                              in_=Wr_ext[:, :, :].rearrange("r d f -> d r f"))
            WrT_sb = constp.tile([D, R, D], BF16, tag="wrt")     # [f, r, d]
            nc.sync.dma_start(out=WrT_sb[:],
                              in_=WrT_ext[:, :, :].rearrange("r f d -> f r d"))
            erow_sb = constp.tile([1, R, D], BF16, tag="er")
            nc.sync.dma_start(out=erow_sb[:], in_=erow_ext[None, :, :])
            W1T_sb = constp.tile([D, L, D], BF16, tag="w1t")
            nc.sync.dma_start(out=W1T_sb[:],
                              in_=W1T_ext[:, :, :].rearrange("l j i -> j l i"))
            W2T_sb = constp.tile([D, L, D], BF16, tag="w2t")
            nc.sync.dma_start(out=W2T_sb[:],
                              in_=W2T_ext[:, :, :].rearrange("l j i -> j l i"))
            ones_row = constp.tile([1, 128], BF16, tag="ones")
            nc.vector.memset(ones_row[:], 1.0)

            hT = hTp.tile([D, NLOC], BF16, tag="hT")

            Vp_cm = tc.tile_pool(name="Vp", bufs=WINDOWS + 1)
            Vp = Vp_cm.__enter__()

            def window_phase(li):
                if li == 0:
                    nc.sync.dma_start(out=hT[:], in_=h0locT_ext[:, :])
                else:
                    for k in range(4):
                        r0 = CHUNK_W[k][0] * 128
                        r1 = CHUNK_W[k][1] * 128
                        nc.sync.dma_start_transpose(out=hT[:, r0:r1],
                                                    in_=hnb1c[k][:, :])
                V_ws = []
                with (
                    tc.tile_pool(name=f"win_ps{li}", bufs=2,
                                 space="PSUM") as win_ps,
                    tc.tile_pool(name=f"twtp{li}", bufs=2) as twtp,
                ):
                    for w in range(WINDOWS):
                        twt_ps = win_ps.tile([128, R * 128], F32, tag="wps")
                        for r in range(R):
                            sl = slice(r * 128, (r + 1) * 128)
                            nc.tensor.matmul(out=twt_ps[:, sl],
                                             lhsT=Wr_sb[:, r, :],
                                             rhs=hT[:, w * 128:(w + 1) * 128],
                                             start=True, stop=False)
                            nc.tensor.matmul(out=twt_ps[:, sl],
                                             lhsT=erow_sb[:, r, :],
                                             rhs=ones_row[:],
                                             start=False, stop=True)
                        twt_sb = twtp.tile([128, R * 128], BF16, tag="twt")
                        nc.scalar.activation(out=twt_sb[:], in_=twt_ps[:],
                                             func=AF.Tanh)
                        v_ps2 = win_ps.tile([128, R * 128], F32, tag="wps")
                        for r in range(R):
                            sl = slice(r * 128, (r + 1) * 128)
                            nc.tensor.matmul(out=v_ps2[:, sl],
                                             lhsT=twt_sb[:, sl],
                                             rhs=WrT_sb[:, r, :],
                                             start=True, stop=True)
                        V_w = Vp.tile([128, R * 128], BF16, tag="V")
                        nc.scalar.activation(out=V_w[:], in_=v_ps2[:],
                                             func=AF.Copy)
                        V_ws.append(V_w)
                return V_ws

            def edge_phase(li, V_ws):
                with (
                    tc.tile_pool(name=f"edge{li}", bufs=2) as edgep,
                    tc.tile_pool(name=f"grp{li}", bufs=4) as grpp,
                    tc.tile_pool(name=f"v_ps{li}", bufs=2,
                                 space="PSUM") as v_psp,
                    tc.tile_pool(name=f"hu_ps{li}", bufs=2,
                                 space="PSUM") as hu_psp,
                    tc.tile_pool(name=f"wout{li}", bufs=2) as outp,
                ):
                    for w in range(WINDOWS):
                        T_w = int(TW[w])
                        TA_w = int(TA[w])
                        NR_w = int(NRW[w])
                        truns = tile_runs[w]
                        V_w = V_ws[w]

                        hs_t = edgep.tile([128, TMAX, 128], BF16, tag="hs")
                        if li == 0:
                            nc.sync.dma_start(
                                out=hs_t[:, 0:T_w, :],
                                in_=hs1_ext[:, int(OT[w]):int(OT[w]) + T_w, :])
                        else:
                            idx_t = edgep.tile([128, 8 * TMAX], I16, tag="ix")
                            c0 = int(OT[w]) * 8
                            nc.sync.dma_start(
                                out=idx_t[:, 0:8 * T_w],
                                in_=idx_ext[:, c0:c0 + 8 * T_w])
                            # pass A then pass B, <=CHUNK rows per call
                            for (tile0, ntile, base) in (
                                    (0, TA_w, 0), (TA_w, T_w - TA_w, HALF)):
                                tab = (hnb_all[0:HALF, :] if base == 0
                                       else hnb_all[HALF:, :])
                                done = 0
                                while done < ntile * 128:
                                    n = min(CHUNK, ntile * 128 - done)
                                    td = tile0 + done // 128
                                    nc.gpsimd.dma_gather(
                                        out_ap=hs_t[:, td:td + n // 128, :],
                                        in_ap=tab,
                                        idxs_ap=idx_t[:, tile0 * 8 + done // 16:
                                                      tile0 * 8 + (done + n) // 16],
                                        num_idxs=n, num_idxs_reg=n,
                                        elem_size=D)
                                    done += n

                        oA_t = edgep.tile([128, NRMAX, 128], FP8, tag="oa")
                        nc.sync.dma_start(
                            out=oA_t[:, 0:NR_w, :],
                            in_=otnA_ext[:, int(OTR[w]):int(OTR[w]) + NR_w, :])
                        oE_t = edgep.tile([128, TMAX, 128], FP8, tag="oe")
                        nc.sync.dma_start(
                            out=oE_t[:, 0:T_w, :],
                            in_=ote_ext[:, int(OT[w]):int(OT[w]) + T_w, :])

                        hu_ps = hu_psp.tile([128, D + 1], F32, tag="hu")
                        ngroups = -(-T_w // GROUP)
                        for g in range(ngroups):
                            j0 = g * GROUP
                            G = min(GROUP, T_w - j0)
                            v_ps = v_psp.tile([128, GROUP, 128], F32,
                                              tag="vps")
                            rbase = sum(len(truns[j]) for j in range(j0))
                            rpos = rbase
                            for j in range(j0, j0 + G):
                                rels = truns[j]
                                for k, rr in enumerate(rels):
                                    nc.tensor.matmul(
                                        out=v_ps[:, j - j0, :],
                                        lhsT=oA_t[:, rpos, :],
                                        rhs=V_w[:, rr * 128:(rr + 1) * 128],
                                        start=(k == 0),
                                        stop=(k == len(rels) - 1))
                                    rpos += 1
                            prod = grpp.tile([128, GROUP, 128], BF16,
                                             tag="prod")
                            nc.vector.tensor_tensor(
                                out=prod[:, 0:G, :], in0=v_ps[:, 0:G, :],
                                in1=hs_t[:, j0:j0 + G, :], op=OP.mult)
                            att = grpp.tile([128, GROUP], BF16, tag="att")
                            with nc.allow_low_precision(
                                    reason="attn logits tolerate bf16"):
                                nc.vector.tensor_reduce(
                                    out=att[:, 0:G], in_=prod[:, 0:G, :],
                                    axis=mybir.AxisListType.X, op=OP.add)
                            ex = grpp.tile([128, GROUP], F32, tag="ex")
                            nc.scalar.activation(out=ex[:, 0:G],
                                                 in_=att[:, 0:G], func=AF.Exp)
                            msg = grpp.tile([128, GROUP, D + 1], BF16,
                                            tag="msg")
                            for j in range(G):
                                nc.vector.tensor_scalar(
                                    out=msg[:, j, 0:D],
                                    in0=hs_t[:, j0 + j, :],
                                    scalar1=ex[:, j:j + 1], scalar2=None,
                                    op0=OP.mult)
                            nc.vector.tensor_copy(out=msg[:, 0:G, D:D + 1],
                                                  in_=ex[:, 0:G, None])
                            for j in range(G):
                                nc.tensor.matmul(
                                    out=hu_ps[:, :],
                                    lhsT=oE_t[:, j0 + j, :],
                                    rhs=msg[:, j, :],
                                    start=(j0 + j == 0),
                                    stop=(j0 + j == T_w - 1))

                        # window epilogue: hnb = hU / max(s, eps)
                        s_cl = outp.tile([128, 1], F32, tag="scl")
                        nc.vector.tensor_scalar(out=s_cl[:],
                                                in0=hu_ps[:, D:D + 1],
                                                scalar1=1e-20, scalar2=None,
                                                op0=OP.max)
                        s_inv = outp.tile([128, 1], F32, tag="sinv")
                        nc.vector.reciprocal(out=s_inv[:], in_=s_cl[:])
                        hnb_bf_w = outp.tile([128, D], BF16, tag="hnbbf")
                        nc.scalar.activation(out=hnb_bf_w[:],
                                             in_=hu_ps[:, 0:D],
                                             func=AF.Copy, scale=s_inv[:, :1])
                        if li == 0:
                            k = _chunk_of_window(w)
                            r0 = (w - CHUNK_W[k][0]) * 128
                            nc.sync.dma_start(
                                out=hnb1c[k][r0:r0 + 128, :], in_=hnb_bf_w[:])
                            if w == CHUNK_W[k][1] - 1:
                                nc.gpsimd.collective_compute(
                                    "AllGather", OP.bypass,
                                    replica_groups=[list(range(NCORES))],
                                    ins=[hnb1c[k][:, :]],
                                    outs=[hnb_all[CUMROWS[k]:CUMROWS[k + 1],
                                                  :]])
                        else:
                            nc.sync.dma_start(
                                out=hnb2bf[w * 128:(w + 1) * 128, :],
                                in_=hnb_bf_w[:])

            V1 = window_phase(0)
            edge_phase(0, V1)
            V2 = window_phase(1)          # overlaps AllGather tail
            edge_phase(1, V2)
            Vp_cm.__exit__(None, None, None)

            # ---------------- epilogue ----------------
            with (
                tc.tile_pool(name="ep", bufs=3) as ep,
                tc.tile_pool(name="epT", bufs=1) as epT,
                tc.tile_pool(name="ep_ps", bufs=2, space="PSUM") as ep_ps,
            ):
                # hT currently holds hnb1T (layer 2's input). Build hnb2T.
                h0T = epT.tile([D, NLOC], BF16, tag="h0T")
                nc.sync.dma_start(out=h0T[:], in_=h0locT_ext[:, :])
                h2T = epT.tile([D, NLOC], BF16, tag="h2T")
                nc.sync.dma_start_transpose(out=h2T[:], in_=hnb2bf[:, :])

                aT = epT.tile([D, L, NLOC], BF16, tag="aT")
                nc.vector.tensor_tensor(out=aT[:, 0, :], in0=h0T[:],
                                        in1=hT[:], op=OP.add)
                nc.vector.tensor_tensor(out=aT[:, 1, :], in0=hT[:],
                                        in1=h2T[:], op=OP.add)
                mT = epT.tile([D, L, NLOC], BF16, tag="mT")
                nc.vector.tensor_tensor(out=mT[:, 0, :], in0=h0T[:],
                                        in1=hT[:], op=OP.mult)
                nc.vector.tensor_tensor(out=mT[:, 1, :], in0=hT[:],
                                        in1=h2T[:], op=OP.mult)

                for w in range(WINDOWS):
                    sl = slice(w * 128, (w + 1) * 128)
                    o_w = ep.tile([128, 2 * D], BF16, tag="ow")
                    for li in range(L):
                        ps1 = ep_ps.tile([128, D], F32, tag="ps1")
                        nc.tensor.matmul(out=ps1[:], lhsT=aT[:, li, sl],
                                         rhs=W1T_sb[:, li, :],
                                         start=True, stop=True)
                        ps2 = ep_ps.tile([128, D], F32, tag="ps2")
                        nc.tensor.matmul(out=ps2[:], lhsT=mT[:, li, sl],
                                         rhs=W2T_sb[:, li, :],
                                         start=True, stop=True)
                        lr1 = ep.tile([128, D], F32, tag="lr1")
                        nc.scalar.activation(out=lr1[:], in_=ps1[:],
                                             func=AF.Lrelu, alpha=NEG_SLOPE)
                        lr2 = ep.tile([128, D], F32, tag="lr2")
                        nc.scalar.activation(out=lr2[:], in_=ps2[:],
                                             func=AF.Lrelu, alpha=NEG_SLOPE)
                        nc.vector.tensor_tensor(
                            out=o_w[:, li * D:(li + 1) * D], in0=lr1[:],
                            in1=lr2[:], op=OP.add)
                    nc.sync.dma_start(out=out_ext[sl, :], in_=o_w[:])

    nc.compile()
    return nc


def kernel(**inputs):
    plan, in_maps = _host_prep(**inputs)
    key = tuple(plan["TW"].tolist())
    if _CACHE.get("key") != key:
        _CACHE["nc"] = _build_nc(plan)
        _CACHE["key"] = key
    nc = _CACHE["nc"]
    res = run_bass_kernel_spmd(nc, in_maps, core_ids=list(range(NCORES)))

    node_ids = np.asarray(inputs["node_ids"]).astype(np.int64)
    h0 = np.asarray(inputs["entity_table"], np.float32)[node_ids]
    out12 = np.concatenate(
        [res.results[c]["out12"][:NPC] for c in range(NCORES)],
        axis=0).astype(np.float32)
    return np.concatenate([h0, out12], axis=1)
